# revision 30
# baseline (speedup 1.0000x reference)
"""Trainium2 Bass kernel for a single-head causal attention block.

Reference computation (per batch element b):
    q = X[b] @ Wq.T ; k = X[b] @ Wk.T ; v = X[b] @ Wv.T        # [S, H]
    aff = softmax(causal_mask(q @ k.T / sqrt(D)))              # [S, S]
    out[b] = aff @ v                                           # [S, H]

Sharding: data-parallel over batch — 8 batch elements, 8 NeuronCores,
one batch element per core. Weights replicated.

Per-core layout strategy ("transposed attention"):
  - Host supplies X^T pre-tiled [p, w, c, s] so every DMA is contiguous.
  - K^T/Q^T are produced by one M=128 matmul pass (lhsT = [Wk; Wq*scale]
    chunks), V^T by an M=64 pass; V^T is PE-transposed into [sk, 64] tiles
    augmented with a ones column (col 64).
  - Attention works on aff^T blocks [sk=128, sq=512]: exp() on ScalarE,
    causal zeroing via DVE mask-mul / gpsimd affine_select, then
    out^T [65, 512] += V_aug[k].T @ P^T accumulated over sk blocks.
    Row 64 of out^T is the softmax denominator (ones column).
  - QK matmuls are row-tiled: even block of a pair runs in PE rows 0..63
    and the odd block concurrently in rows 64..127 (K^T/Q^T replicas
    parked at partitions 64..127 via a PE permutation matmul).
  - Causal trimming: diagonal blocks only compute/exp/mask/accumulate
    columns [delta:512]; fully masked blocks are never emitted.
  - out^T (numerator rows 0..63 + denominator row 64) is DMA'd to DRAM
    untransposed and unnormalized; the host does the divide + transpose.
  - Global software pipeline: all windows' attention pairs form one
    stream; the next windows' projection matmuls are injected between
    pairs ("fillers"). V(3) fillers are held back so the exp-bound final
    window still has PE work to overlap with ScalarE.
  - Startup: warmup matmuls from t=0 lift the PE HAM clock-gate while
    prioritized fine-grained DMAs (wkq, then xt0 per-2-chunk across all
    three queue engines) land the first window's data ~2us in.
"""

import sys

if "/opt/trn_rl_repo" not in sys.path:
    sys.path.insert(0, "/opt/trn_rl_repo")

import numpy as np

B, S, D, H = 8, 2048, 1024, 64
N_CORES = 8
W = 512           # sq window width
NW = S // W       # 4 windows
NC_ = D // 128    # 8 d-chunks
NB = S // 128     # 16 sk blocks

XDT_NAME = "bf16"   # X / projection weights / Q^T / K^T
PDT_NAME = "bf16"   # V tiles and exp(aff) (PV matmul operands)

N_WARMUP_MM = 6     # dense warmup matmuls at t=0 (HAM clock-gate lift)
WARM_N = 512        # warmup matmul moving width
W_ORDER = [0, 1, 2, 3]   # attention window processing order
FILL_PER_PAIR = 4        # reservoir items drained after each QK pair
TRIM = True              # causal column trimming on diagonal blocks

_compiled = None


def _build():
    import concourse.mybir as mybir
    import concourse.tile as tile
    from concourse import bacc

    f32 = mybir.dt.float32
    xdt = getattr(mybir.dt, {"bf16": "bfloat16", "f32r": "float32r"}[XDT_NAME])
    pdt = getattr(mybir.dt, {"bf16": "bfloat16", "f32r": "float32r"}[PDT_NAME])

    nc = bacc.Bacc(None, target_bir_lowering=False)

    XT = nc.declare_dram_parameter("XT", [128, NW, NC_, W], xdt, isOutput=False)
    WKQ = nc.declare_dram_parameter("WKQ", [128, NC_, 128], xdt, isOutput=False)
    WV = nc.declare_dram_parameter("WV", [128, NC_, H], xdt, isOutput=False)
    IDT = nc.declare_dram_parameter("IDT", [128, 128], pdt, isOutput=False)
    Y = nc.declare_dram_parameter("Y", [H + 1, NW, W], f32, isOutput=True)

    Exp = mybir.ActivationFunctionType.Exp
    ge = mybir.AluOpType.is_ge

    with tile.TileContext(nc) as tc:
        with (
            tc.tile_pool(name="const", bufs=1) as const,
            tc.tile_pool(name="persist", bufs=1) as persist,
            tc.tile_pool(name="evac", bufs=2) as evac_pool,
            tc.tile_pool(name="pt", bufs=4) as pt_pool,
            tc.tile_pool(name="outp", bufs=2) as out_pool,
            tc.tile_pool(name="ps_kq", bufs=1, space="PSUM") as ps_kq,
            tc.tile_pool(name="ps_vt", bufs=1, space="PSUM") as ps_vt,
            tc.tile_pool(name="ps_tr", bufs=1, space="PSUM") as ps_tr,
            tc.tile_pool(name="ps_aff", bufs=2, space="PSUM") as ps_aff,
            tc.tile_pool(name="ps_out", bufs=1, space="PSUM") as ps_out,
        ):
            # ---- PE warmup: dense matmuls from t=0 so the HAM clock-gate
            # opens (~3.4us sustained busy) while startup DMAs land ----
            scratch = const.tile([128, WARM_N], xdt)
            nc.vector.memset(scratch, 0.0)
            exp_warm = const.tile([128, 2], f32)
            nc.vector.memset(exp_warm[:, 0:1], 0.0)
            # wide warmups for bulk busy-time, then a short-matmul tail so
            # the handoff to the first (DMA-gated) projection matmul never
            # leaves a PE idle gap big enough to re-arm the HAM throttle
            warm = ps_aff.tile([128, 2 * W], f32, tag="aff")
            warm_widths = [WARM_N] * N_WARMUP_MM + [128] * 4
            for i, wn in enumerate(warm_widths):
                nc.tensor.matmul(
                    warm[:, 0:wn], scratch[:, 0:128], scratch[:, 0:wn],
                    start=(i == 0), stop=(i == len(warm_widths) - 1),
                )

            # ---- startup DMAs: each dma_start costs ~0.7us of issue time
            # on its engine and aggregate HBM is ~325 GB/s, so the three
            # queues carry ONLY wkq + xt0 (+ tiny wv/idp) until window 0
            # is resident; xt2/xt3 are gated behind xt0 so they cannot
            # steal bandwidth from the critical window ----
            wkq_sb = const.tile([128, NC_, 128], xdt)
            xt_tiles = []
            for w in range(NW):
                xt_tiles.append(persist.tile([128, NC_, W], xdt, tag=f"xt{w}", name=f"xt{w}"))
            wv_sb = const.tile([128, NC_, H], xdt)
            idp = const.tile([128, 128], pdt)

            # sync queue: first wkq chunks gate the very first matmul;
            # per-queue FIFO then serves xt0 tail -> xt1 -> xt2 -> xt3 in
            # exactly criticality order without stealing bandwidth early
            nc.sync.dma_start(out=wkq_sb[:, 0:2, :], in_=WKQ[:, 0:2, :])
            nc.sync.dma_start(out=wkq_sb[:, 2:8, :], in_=WKQ[:, 2:8, :])
            nc.sync.dma_start(out=xt_tiles[0][:, 6:8, :], in_=XT[:, 0, 6:8, :])
            nc.sync.dma_start(out=xt_tiles[1][:, 4:8, :], in_=XT[:, 1, 4:8, :])
            nc.sync.dma_start(out=xt_tiles[2][:, 0:4, :], in_=XT[:, 2, 0:4, :])
            nc.sync.dma_start(out=xt_tiles[2][:, 4:8, :], in_=XT[:, 2, 4:8, :])
            nc.sync.dma_start(out=xt_tiles[3][:, 0:4, :], in_=XT[:, 3, 0:4, :])
            nc.sync.dma_start(out=xt_tiles[3][:, 4:8, :], in_=XT[:, 3, 4:8, :])
            # scalar queue: xt0 low chunks (first matmul dependency)
            nc.scalar.dma_start(out=xt_tiles[0][:, 0:1, :], in_=XT[:, 0, 0:1, :])
            nc.scalar.dma_start(out=xt_tiles[0][:, 1:2, :], in_=XT[:, 0, 1:2, :])
            nc.scalar.dma_start(out=xt_tiles[0][:, 2:4, :], in_=XT[:, 0, 2:4, :])
            nc.scalar.dma_start(out=xt_tiles[1][:, 0:4, :], in_=XT[:, 1, 0:4, :])
            # gpsimd queue: the two small constants, then a third lane
            # for xt0's middle chunks
            nc.gpsimd.dma_start(out=wv_sb, in_=WV[:, :, :])
            nc.gpsimd.dma_start(out=idp, in_=IDT[:, :])
            nc.gpsimd.dma_start(out=xt_tiles[0][:, 4:6, :], in_=XT[:, 0, 4:6, :])

            # preload the Exp activation table while startup DMAs run
            nc.scalar.activation(out=exp_warm[:, 1:2], in_=exp_warm[:, 0:1], func=Exp)

            # causal mask strip generated on device: M[p, c] = (c - p >= 384)
            mask_sb = const.tile([128, 896], pdt)
            nc.vector.memset(mask_sb, 1.0)
            nc.gpsimd.affine_select(
                out=mask_sb, in_=mask_sb,
                compare_op=ge, fill=0.0,
                base=-384, pattern=[[1, 896]], channel_multiplier=-1,
            )
            # swap permutation (64-rotated identity) built from idp
            swp_sb = const.tile([128, 128], xdt)
            nc.vector.tensor_copy(swp_sb[:, 0:64], idp[:, 64:128])
            nc.vector.tensor_copy(swp_sb[:, 64:128], idp[:, 0:64])


            # kq_all rows 0..63 = K^T, rows 64..127 = Q^T (as projected).
            # kqswap = 64-rotated copy (rows 0..63 = Q^T, rows 64..127 = K^T)
            # produced by a PE permutation matmul — SBUF-SBUF DMA shifts have
            # multi-us latency and serialized the whole attention stream.
            kq_all = persist.tile([128, S], xdt)
            kqswap = persist.tile([128, S], xdt)
            v_aug = persist.tile([128, NB, H + 1], pdt)  # V blocks + ones col
            ones_sb = const.tile([128, 1], f32)
            nc.vector.memset(ones_sb, 1.0)
            for k in range(NB):
                nc.vector.tensor_copy(v_aug[:, k, H : H + 1], ones_sb)

            # ---------------- projection emission (as filler items) ----
            def kq_items(w, order=None):
                win = slice(w * W, (w + 1) * W)
                xt_w = xt_tiles[w]
                order = order if order is not None else list(range(NC_))
                pkq_box = {}

                def mk_mm(c, st, sp):
                    def f():
                        if st:
                            pkq_box["t"] = ps_kq.tile([128, W], f32, tag="kq", name="pkq")
                        nc.tensor.matmul(
                            pkq_box["t"], wkq_sb[:, c, :], xt_w[:, c, :],
                            start=st, stop=sp,
                        )
                    return f

                def evac():
                    nc.vector.tensor_copy(kq_all[:, win], pkq_box["t"])

                def swap():
                    psw = ps_kq.tile([128, W], f32, tag="kq", name="psw")
                    nc.tensor.matmul(
                        psw, swp_sb, kq_all[:, win], start=True, stop=True
                    )
                    nc.vector.tensor_copy(kqswap[:, win], psw)

                return [mk_mm(c, i == 0, i == NC_ - 1)
                        for i, c in enumerate(order)] + [evac, swap]

            def v_items(w, order=None):
                xt_w = xt_tiles[w]
                order = order if order is not None else list(range(NC_))
                box = {}

                def mk_mm(c, st, sp):
                    def f():
                        if st:
                            box["p"] = ps_vt.tile([64, W], f32, tag="vt", name="pvt")
                        nc.tensor.matmul(
                            box["p"], wv_sb[:, c, :], xt_w[:, c, :],
                            start=st, stop=sp,
                        )
                    return f

                def evac():
                    box["v"] = evac_pool.tile([64, W], pdt, tag="vtmp", name="vt_tmp")
                    nc.vector.tensor_copy(box["v"], box["p"])

                def mk_tr(t):
                    def f():
                        if t == 0:
                            box["tr"] = ps_tr.tile([128, 4, H + 2], pdt, tag="tr", name="ptr")
                        nc.tensor.transpose(
                            box["tr"][:, t, 0:H],
                            box["v"][:, t * 128 : (t + 1) * 128],
                            idp[0:64, 0:64],
                        )
                    return f

                def vcopy():
                    nc.vector.tensor_copy(
                        v_aug[:, 4 * w : 4 * w + 4, 0:H], box["tr"][:, :, 0:H]
                    )

                return [mk_mm(c, i == 0, i == NC_ - 1)
                        for i, c in enumerate(order)] + [evac] \
                    + [mk_tr(t) for t in range(4)] + [vcopy]

            def v_items_pair(wlo, whi):
                # V projections have M=64 — half the PE array idle.  Run
                # two windows' V matmuls column-tiled (array cols 0:63 /
                # 64:127) so both accumulate concurrently for the price
                # of one pass.  whi's data stays at partitions 64:128
                # through psum->sbuf evac (engine copies cannot shift
                # partitions); its transposes use the 64:128 diagonal of
                # the identity.
                box = {}

                def mk_mm(c, st, sp):
                    def f():
                        if st:
                            box["p"] = ps_vt.tile([128, W], f32, tag="vt", name="pvt2")
                        nc.tensor.matmul(
                            box["p"][0:64, :], wv_sb[:, c, :],
                            xt_tiles[wlo][:, c, :], start=st, stop=sp,
                        )
                        nc.tensor.matmul(
                            box["p"][64:128, :], wv_sb[:, c, :],
                            xt_tiles[whi][:, c, :], start=st, stop=sp,
                            tile_position=(0, 64),
                        )
                    return f

                def evac():
                    box["v"] = evac_pool.tile([128, W], pdt, tag="vtmp2", name="vt_pair")
                    nc.vector.tensor_copy(box["v"], box["p"])

                def mk_tr(w, half, t):
                    def f():
                        key = f"tr{half}"
                        if t == 0:
                            box[key] = ps_tr.tile([128, 4, H + 2], pdt, tag="tr", name=f"ptr{half}")
                        lo = 64 * half
                        nc.tensor.transpose(
                            box[key][:, t, 0:H],
                            box["v"][lo : lo + 64, t * 128 : (t + 1) * 128],
                            idp[lo : lo + 64, lo : lo + 64],
                        )
                    return f

                def mk_vcopy(w, half):
                    def f():
                        nc.vector.tensor_copy(
                            v_aug[:, 4 * w : 4 * w + 4, 0:H],
                            box[f"tr{half}"][:, :, 0:H],
                        )
                    return f

                items = [mk_mm(c, i == 0, i == NC_ - 1) for i, c in enumerate(range(NC_))]
                items.append(evac)
                for half, w in ((0, wlo), (1, whi)):
                    items += [mk_tr(w, half, t) for t in range(4)]
                    items.append(mk_vcopy(w, half))
                return items

            def out_evac(w, pouts):
                # emitted IMMEDIATELY at window close: frees the single
                # ps_out bank so the next window's PV WAR-dep is tracked
                box = {}
                box["oT"] = out_pool.tile([H + 1, W], f32, tag="oT", name="oT")
                nc.vector.tensor_copy(box["oT"], pouts.pop(w))
                return box

            def out_items(w, box):
                # unnormalized out^T straight to DRAM; host divides row 64
                def dma():
                    nc.sync.dma_start(out=Y[:, w, :], in_=box["oT"])

                return [dma]

            # ---------------- reservoir of interleavable work ----------
            reservoir = []  # list of (tag, closure)

            def add_group(tag, items):
                for it in items:
                    reservoir.append((tag, it))

            def add_group_front(tag, items):
                # K/Q projections gate the next window's attention start:
                # they must drain through fills before v/out leftovers
                for j, it in enumerate(items):
                    reservoir.insert(j, (tag, it))

            def deferred(tag):
                # V(3) is only consumed by window 3's PV stream; hold it
                # back so the exp-bound last window keeps PE fillers
                return tag == ("v", 3)

            def fill(n):
                for _ in range(n):
                    pick = None
                    for j, (tag, it) in enumerate(reservoir):
                        if not deferred(tag):
                            pick = j
                            break
                    if pick is None:
                        if not reservoir:
                            return
                        pick = 0
                    reservoir.pop(pick)[1]()

            def flush(pred):
                keep = []
                for tag, it in reservoir:
                    if pred(tag):
                        it()
                    else:
                        keep.append((tag, it))
                reservoir[:] = keep

            # ---------------- attention pair stream --------------------
            def lo_of(w, k):
                d = 128 * k - W * w
                return max(0, d) if TRIM else 0

            def emit_qk_exp(w, p, pts):
                # trimmed halves packed around the bank boundary: h0 at
                # [lo0:W] (end of psum bank 0), h1 at [W:W+n1] (start of
                # bank 1) — contiguous so ONE exp covers the pair
                # (activations carry ~260ns fixed overhead per instr)
                # while the two concurrent row-tiled matmuls never write
                # the same psum bank.
                win0 = w * W
                k0, k1 = 2 * p, 2 * p + 1
                paff = ps_aff.tile([128, 2 * W], f32, tag="aff")
                pt = pt_pool.tile([128, 2 * W], pdt, tag="pt")
                lo0, lo1 = lo_of(w, k0), lo_of(w, k1)
                n1 = W - lo1
                pts[(w, p)] = (pt, lo0, n1)
                # row-tiled pair: block k0 in PE rows 0..63,
                # block k1 concurrently in rows 64..127
                nc.tensor.matmul(
                    paff[:, lo0:W],
                    kq_all[0:64, k0 * 128 : (k0 + 1) * 128],
                    kqswap[0:64, win0 + lo0 : win0 + W],
                    start=True, stop=True,
                )
                nc.tensor.matmul(
                    paff[:, W : W + n1],
                    kqswap[64:128, k1 * 128 : (k1 + 1) * 128],
                    kq_all[64:128, win0 + lo1 : win0 + W],
                    start=True, stop=True,
                )
                nc.scalar.activation(
                    out=pt[:, lo0 : W + n1], in_=paff[:, lo0 : W + n1], func=Exp
                )
                if k1 >= 4 * w:  # pair contains (partially) masked blocks
                    for h, (k, lo, off) in ((0, (k0, lo0, 0)), (1, (k1, lo1, W))):
                        delta = 128 * k - W * w
                        if delta > -128:
                            lom = max(0, delta) if TRIM else 0
                            him = min(W, delta + 128) if TRIM else W
                            if h == 0:
                                half = pt[:, lom:him]
                                # zero where sk > sq via 0/1 mask multiply
                                nc.vector.tensor_mul(
                                    half, half,
                                    mask_sb[:, 384 - delta + lom : 384 - delta + him],
                                )
                            else:
                                half = pt[:, W + lom - lo : W + him - lo]
                                # same predicate on the gpsimd engine so
                                # the two halves mask in parallel
                                nc.gpsimd.affine_select(
                                    out=half, in_=half,
                                    compare_op=ge, fill=0.0,
                                    base=lom - delta,
                                    pattern=[[1, him - lom]],
                                    channel_multiplier=-1,
                                )

            def emit_pv(w, p, pts, pouts):
                if p == 0:
                    pouts[w] = ps_out.tile([H + 1, W], f32, tag="out", name="pout")
                pout = pouts[w]
                nblk = 4 * w + 4
                pt, lo0, n1 = pts.pop((w, p))
                for h, (k, lo, off) in (
                    (0, (2 * p, lo0, lo0)),
                    (1, (2 * p + 1, W - n1, W)),
                ):
                    nc.tensor.matmul(
                        pout[:, lo:W],
                        v_aug[:, k, :],
                        pt[:, off : off + (W - lo)],
                        start=(k == 0), stop=(k == nblk - 1),
                    )

            # ---------------- main schedule ----------------------------
            # proj(0) emitted directly, KQ/V chunk matmuls interleaved so
            # the PE tracks the DMA chunk-arrival pace without idling;
            # later windows' projections ride the reservoir.
            # chunk consumption follows the DMA arrival order:
            # c0,c1 (scalar head), c6,c7 (sync, right after wkq),
            # then c2,c3 (scalar), c4,c5 (gpsimd third lane)
            ORDER0 = [0, 1, 6, 7, 2, 3, 4, 5]
            kq0 = kq_items(0, ORDER0)
            v0 = v_items(0, ORDER0)
            for c in range(NC_):
                kq0[c]()
                v0[c]()
            for it in kq0[NC_:]:   # kq evac + swap
                it()
            for it in v0[NC_:]:    # v evac + transposes + vcopy
                it()
            nxt = {0: 1, 1: 2, 2: 3, 3: None}  # filler proj after window
            fills = {0: 4, 1: 4, 2: 4, 3: 4}
            pts, pouts = {}, {}
            pairs = [(w, p) for w in W_ORDER for p in range(2 * w + 2)]
            emitted_proj = {0}
            oT3 = None
            for i, (w, p) in enumerate(pairs):
                if p == 0:
                    # barrier: this window's K/Q projection must be emitted
                    flush(lambda t: t == ("kq", w) or t == ("out",))
                emit_qk_exp(w, p, pts)
                if p == 0 and nxt[w] is not None and nxt[w] not in emitted_proj:
                    u = nxt[w]
                    emitted_proj.add(u)
                    add_group_front(("kq", u), kq_items(u))
                    if u == 2:
                        # V(1)+V(2) run column-tiled as one pass; tag as
                        # ("v", 1) so any flush needing V1 forces both
                        add_group(("v", 1), v_items_pair(1, 2))
                    elif u != 1:
                        add_group(("v", u), v_items(u))
                fill(fills[w])
                if i > 0:
                    wp, pp = pairs[i - 1]
                    # barrier: PV(wp,pp) reads v_aug blocks 2pp,2pp+1
                    vneed = (2 * pp + 1) // 4
                    flush(lambda t: t[0] == "v" and t[1] <= vneed)
                    emit_pv(wp, pp, pts, pouts)
                    if pp == 2 * wp + 1:  # closed window wp
                        box = out_evac(wp, pouts)
                        add_group(("out",), out_items(wp, box))
                    elif (wp, pp) == (3, 6):
                        # cols 0:256 of window 3's out^T are final after
                        # pair 6 (pair 7's diagonal blocks only touch
                        # 256:512): ship them before the last pair so the
                        # end-of-kernel DMA+receipt only covers half
                        oT3 = out_pool.tile([H + 1, W], f32, tag="oT", name="oT3")
                        nc.vector.tensor_copy(oT3[:, 0:256], pouts[3][:, 0:256])
                        nc.sync.dma_start(out=Y[:, 3, 0:256], in_=oT3[:, 0:256])
            wl, pl = pairs[-1]
            flush(lambda t: t[0] == "v" and t[1] <= (2 * pl + 1) // 4)
            emit_pv(wl, pl, pts, pouts)
            flush(lambda t: True)
            nc.vector.tensor_copy(oT3[:, 256:512], pouts.pop(wl)[:, 256:512])
            nc.sync.dma_start(out=Y[:, wl, 256:512], in_=oT3[:, 256:512])

    nc.finalize()
    return nc


def _np_dt(name):
    if name == "bf16":
        import ml_dtypes

        return ml_dtypes.bfloat16
    return np.float32


def _host_inputs(X, Wk, Wq, Wv):
    """Per-core input dicts (host-side sharding + layout prep)."""
    xnp = _np_dt(XDT_NAME)
    scale = 1.0 / np.sqrt(np.float32(D))
    wkq = np.concatenate([Wk, Wq * scale], axis=0).T  # [D, 128]
    wkq = np.ascontiguousarray(
        wkq.reshape(NC_, 128, 128).transpose(1, 0, 2)
    ).astype(xnp)  # [p, c, m]
    wv = np.ascontiguousarray(
        Wv.T.reshape(NC_, 128, H).transpose(1, 0, 2)
    ).astype(xnp)  # [p, c, h]
    pnp = _np_dt(PDT_NAME)
    idt = np.eye(128, dtype=np.float32).astype(pnp)

    in_maps = []
    for b in range(N_CORES):
        xt = np.ascontiguousarray(
            X[b].T.reshape(NC_, 128, NW, W).transpose(1, 2, 0, 3)
        ).astype(xnp)  # [p, w, c, s]
        in_maps.append({"XT": xt, "WKQ": wkq, "WV": wv, "IDT": idt})
    return in_maps


def _unshard(results):
    """Device Y is unnormalized out^T [h(+denom), w, s]; divide + transpose."""
    outs = []
    for i in range(N_CORES):
        y = results[i]["Y"]  # [65, NW, W] f32
        o = y[:H] / y[H : H + 1]  # [64, 4, 512]
        outs.append(np.ascontiguousarray(o.transpose(1, 2, 0)).reshape(S, H))
    return np.stack(outs, axis=0).astype(np.float32)


def kernel(X, Wk, Wq, Wv):
    global _compiled
    from concourse.bass_utils import run_bass_kernel_spmd

    if _compiled is None:
        _compiled = _build()
    in_maps = _host_inputs(
        np.asarray(X, dtype=np.float32),
        np.asarray(Wk, dtype=np.float32),
        np.asarray(Wq, dtype=np.float32),
        np.asarray(Wv, dtype=np.float32),
    )
    res = run_bass_kernel_spmd(_compiled, in_maps, list(range(N_CORES)))
    return _unshard(res.results)


# revision 33
# speedup vs baseline: 1.0160x; 1.0160x over previous
"""Trainium2 Bass kernel for a single-head causal attention block.

Reference computation (per batch element b):
    q = X[b] @ Wq.T ; k = X[b] @ Wk.T ; v = X[b] @ Wv.T        # [S, H]
    aff = softmax(causal_mask(q @ k.T / sqrt(D)))              # [S, S]
    out[b] = aff @ v                                           # [S, H]

Sharding: data-parallel over batch — 8 batch elements, 8 NeuronCores,
one batch element per core. Weights replicated.

Per-core layout strategy ("transposed attention"):
  - Host supplies X^T pre-tiled [p, w, c, s] so every DMA is contiguous.
  - K^T/Q^T are produced by one M=128 matmul pass (lhsT = [Wk; Wq*scale]
    chunks), V^T by an M=64 pass; V^T is PE-transposed into [sk, 64] tiles
    augmented with a ones column (col 64).
  - Attention works on aff^T blocks [sk=128, sq=512]: exp() on ScalarE,
    causal zeroing via DVE mask-mul / gpsimd affine_select, then
    out^T [65, 512] += V_aug[k].T @ P^T accumulated over sk blocks.
    Row 64 of out^T is the softmax denominator (ones column).
  - QK matmuls are row-tiled: even block of a pair runs in PE rows 0..63
    and the odd block concurrently in rows 64..127 (K^T/Q^T replicas
    parked at partitions 64..127 via a PE permutation matmul).
  - Causal trimming: diagonal blocks only compute/exp/mask/accumulate
    columns [delta:512]; fully masked blocks are never emitted.
  - out^T (numerator rows 0..63 + denominator row 64) is DMA'd to DRAM
    untransposed and unnormalized; the host does the divide + transpose.
  - Global software pipeline: all windows' attention pairs form one
    stream; the next windows' projection matmuls are injected between
    pairs ("fillers"). V(3) fillers are held back so the exp-bound final
    window still has PE work to overlap with ScalarE.
  - Startup: warmup matmuls from t=0 lift the PE HAM clock-gate while
    prioritized fine-grained DMAs (wkq, then xt0 per-2-chunk across all
    three queue engines) land the first window's data ~2us in.
"""

import sys

if "/opt/trn_rl_repo" not in sys.path:
    sys.path.insert(0, "/opt/trn_rl_repo")

import numpy as np

B, S, D, H = 8, 2048, 1024, 64
N_CORES = 8
W = 512           # sq window width
NW = S // W       # 4 windows
NC_ = D // 128    # 8 d-chunks
NB = S // 128     # 16 sk blocks

XDT_NAME = "bf16"   # X / projection weights / Q^T / K^T
PDT_NAME = "bf16"   # V tiles and exp(aff) (PV matmul operands)

N_WARMUP_MM = 6     # dense warmup matmuls at t=0 (HAM clock-gate lift)
WARM_N = 512        # warmup matmul moving width
W_ORDER = [0, 1, 2, 3]   # attention window processing order
FILL_PER_PAIR = 4        # reservoir items drained after each QK pair
TRIM = True              # causal column trimming on diagonal blocks

_compiled = None


def _build():
    import concourse.mybir as mybir
    import concourse.tile as tile
    from concourse import bacc

    f32 = mybir.dt.float32
    xdt = getattr(mybir.dt, {"bf16": "bfloat16", "f32r": "float32r"}[XDT_NAME])
    pdt = getattr(mybir.dt, {"bf16": "bfloat16", "f32r": "float32r"}[PDT_NAME])

    nc = bacc.Bacc(None, target_bir_lowering=False)

    XT = nc.declare_dram_parameter("XT", [128, NW, NC_, W], xdt, isOutput=False)
    WKQ = nc.declare_dram_parameter("WKQ", [128, NC_, 128], xdt, isOutput=False)
    WV = nc.declare_dram_parameter("WV", [128, NC_, H], xdt, isOutput=False)
    IDT = nc.declare_dram_parameter("IDT", [128, 128], pdt, isOutput=False)
    Y = nc.declare_dram_parameter("Y", [H + 1, NW, W], f32, isOutput=True)

    Exp = mybir.ActivationFunctionType.Exp
    ge = mybir.AluOpType.is_ge

    with tile.TileContext(nc) as tc:
        with (
            tc.tile_pool(name="const", bufs=1) as const,
            tc.tile_pool(name="persist", bufs=1) as persist,
            tc.tile_pool(name="evac", bufs=2) as evac_pool,
            tc.tile_pool(name="pt", bufs=4) as pt_pool,
            tc.tile_pool(name="outp", bufs=2) as out_pool,
            tc.tile_pool(name="ps_kq", bufs=1, space="PSUM") as ps_kq,
            tc.tile_pool(name="ps_vt", bufs=1, space="PSUM") as ps_vt,
            tc.tile_pool(name="ps_tr", bufs=1, space="PSUM") as ps_tr,
            tc.tile_pool(name="ps_aff", bufs=2, space="PSUM") as ps_aff,
            tc.tile_pool(name="ps_out", bufs=1, space="PSUM") as ps_out,
        ):
            # ---- PE warmup: dense matmuls from t=0 so the HAM clock-gate
            # opens (~3.4us sustained busy) while startup DMAs land ----
            scratch = const.tile([128, WARM_N], xdt)
            nc.vector.memset(scratch, 0.0)
            exp_warm = const.tile([128, 2], f32)
            nc.vector.memset(exp_warm[:, 0:1], 0.0)
            # wide warmups for bulk busy-time, then a short-matmul tail so
            # the handoff to the first (DMA-gated) projection matmul never
            # leaves a PE idle gap big enough to re-arm the HAM throttle
            # run the warmup PAST the point where xt0 is fully resident
            # (~14us): chasing individual chunk arrivals leaves PE gaps
            # whenever HBM receipt jitters, and one >1.7us gap re-arms
            # the HAM throttle for 3-7us.  A dependency-free bulk keeps
            # the busy-window unbroken; the projection then runs at
            # guaranteed-warm speed with every chunk already in SBUF.
            warm = ps_aff.tile([128, 2 * W], f32, tag="aff")
            warm_widths = [WARM_N] * 12 + [128] * 8
            for i, wn in enumerate(warm_widths):
                nc.tensor.matmul(
                    warm[:, 0:wn], scratch[:, 0:128], scratch[:, 0:wn],
                    start=(i == 0), stop=(i == len(warm_widths) - 1),
                )

            # ---- startup DMAs: each dma_start costs ~0.7us of issue time
            # on its engine and aggregate HBM is ~325 GB/s, so the three
            # queues carry ONLY wkq + xt0 (+ tiny wv/idp) until window 0
            # is resident; xt2/xt3 are gated behind xt0 so they cannot
            # steal bandwidth from the critical window ----
            wkq_sb = const.tile([128, NC_, 128], xdt)
            xt_tiles = []
            for w in range(NW):
                xt_tiles.append(persist.tile([128, NC_, W], xdt, tag=f"xt{w}", name=f"xt{w}"))
            wv_sb = const.tile([128, NC_, H], xdt)
            idp = const.tile([128, 128], pdt)

            # sync queue: first wkq chunks gate the very first matmul;
            # per-queue FIFO then serves xt0 tail -> xt1 -> xt2 -> xt3 in
            # exactly criticality order without stealing bandwidth early
            nc.sync.dma_start(out=wkq_sb[:, 0:2, :], in_=WKQ[:, 0:2, :])
            nc.sync.dma_start(out=wkq_sb[:, 2:8, :], in_=WKQ[:, 2:8, :])
            nc.sync.dma_start(out=xt_tiles[0][:, 6:8, :], in_=XT[:, 0, 6:8, :])
            nc.sync.dma_start(out=xt_tiles[1][:, 4:8, :], in_=XT[:, 1, 4:8, :])
            nc.sync.dma_start(out=xt_tiles[2][:, 0:4, :], in_=XT[:, 2, 0:4, :])
            nc.sync.dma_start(out=xt_tiles[2][:, 4:8, :], in_=XT[:, 2, 4:8, :])
            nc.sync.dma_start(out=xt_tiles[3][:, 0:4, :], in_=XT[:, 3, 0:4, :])
            nc.sync.dma_start(out=xt_tiles[3][:, 4:8, :], in_=XT[:, 3, 4:8, :])
            # scalar queue: xt0 low chunks (first matmul dependency)
            nc.scalar.dma_start(out=xt_tiles[0][:, 0:1, :], in_=XT[:, 0, 0:1, :])
            nc.scalar.dma_start(out=xt_tiles[0][:, 1:2, :], in_=XT[:, 0, 1:2, :])
            nc.scalar.dma_start(out=xt_tiles[0][:, 2:4, :], in_=XT[:, 0, 2:4, :])
            nc.scalar.dma_start(out=xt_tiles[1][:, 0:4, :], in_=XT[:, 1, 0:4, :])
            # gpsimd queue: the two small constants, then a third lane
            # for xt0's middle chunks
            nc.gpsimd.dma_start(out=wv_sb, in_=WV[:, :, :])
            nc.gpsimd.dma_start(out=idp, in_=IDT[:, :])
            nc.gpsimd.dma_start(out=xt_tiles[0][:, 4:6, :], in_=XT[:, 0, 4:6, :])

            # preload the Exp activation table while startup DMAs run
            nc.scalar.activation(out=exp_warm[:, 1:2], in_=exp_warm[:, 0:1], func=Exp)

            # causal mask strip generated on device: M[p, c] = (c - p >= 384)
            mask_sb = const.tile([128, 896], pdt)
            nc.vector.memset(mask_sb, 1.0)
            nc.gpsimd.affine_select(
                out=mask_sb, in_=mask_sb,
                compare_op=ge, fill=0.0,
                base=-384, pattern=[[1, 896]], channel_multiplier=-1,
            )
            # swap permutation (64-rotated identity) built from idp
            swp_sb = const.tile([128, 128], xdt)
            nc.vector.tensor_copy(swp_sb[:, 0:64], idp[:, 64:128])
            nc.vector.tensor_copy(swp_sb[:, 64:128], idp[:, 0:64])


            # kq_all rows 0..63 = K^T, rows 64..127 = Q^T (as projected).
            # kqswap = 64-rotated copy (rows 0..63 = Q^T, rows 64..127 = K^T)
            # produced by a PE permutation matmul — SBUF-SBUF DMA shifts have
            # multi-us latency and serialized the whole attention stream.
            kq_all = persist.tile([128, S], xdt)
            kqswap = persist.tile([128, S], xdt)
            v_aug = persist.tile([128, NB, H + 1], pdt)  # V blocks + ones col
            ones_sb = const.tile([128, 1], f32)
            nc.vector.memset(ones_sb, 1.0)
            for k in range(NB):
                nc.vector.tensor_copy(v_aug[:, k, H : H + 1], ones_sb)

            # ---------------- projection emission (as filler items) ----
            def kq_items(w, order=None):
                win = slice(w * W, (w + 1) * W)
                xt_w = xt_tiles[w]
                order = order if order is not None else list(range(NC_))
                pkq_box = {}

                def mk_mm(c, st, sp):
                    def f():
                        if st:
                            pkq_box["t"] = ps_kq.tile([128, W], f32, tag="kq", name="pkq")
                        nc.tensor.matmul(
                            pkq_box["t"], wkq_sb[:, c, :], xt_w[:, c, :],
                            start=st, stop=sp,
                        )
                    return f

                def evac():
                    nc.vector.tensor_copy(kq_all[:, win], pkq_box["t"])

                def swap():
                    psw = ps_kq.tile([128, W], f32, tag="kq", name="psw")
                    nc.tensor.matmul(
                        psw, swp_sb, kq_all[:, win], start=True, stop=True
                    )
                    nc.vector.tensor_copy(kqswap[:, win], psw)

                return [mk_mm(c, i == 0, i == NC_ - 1)
                        for i, c in enumerate(order)] + [evac, swap]

            def v_items(w, order=None):
                xt_w = xt_tiles[w]
                order = order if order is not None else list(range(NC_))
                box = {}

                def mk_mm(c, st, sp):
                    def f():
                        if st:
                            box["p"] = ps_vt.tile([64, W], f32, tag="vt", name="pvt")
                        nc.tensor.matmul(
                            box["p"], wv_sb[:, c, :], xt_w[:, c, :],
                            start=st, stop=sp,
                        )
                    return f

                def evac():
                    box["v"] = evac_pool.tile([64, W], pdt, tag="vtmp", name="vt_tmp")
                    nc.vector.tensor_copy(box["v"], box["p"])

                def mk_tr(t):
                    def f():
                        if t == 0:
                            box["tr"] = ps_tr.tile([128, 4, H + 2], pdt, tag="tr", name="ptr")
                        nc.tensor.transpose(
                            box["tr"][:, t, 0:H],
                            box["v"][:, t * 128 : (t + 1) * 128],
                            idp[0:64, 0:64],
                        )
                    return f

                def vcopy():
                    nc.vector.tensor_copy(
                        v_aug[:, 4 * w : 4 * w + 4, 0:H], box["tr"][:, :, 0:H]
                    )

                return [mk_mm(c, i == 0, i == NC_ - 1)
                        for i, c in enumerate(order)] + [evac] \
                    + [mk_tr(t) for t in range(4)] + [vcopy]

            def v_items_pair(wlo, whi):
                # V projections have M=64 — half the PE array idle.  Run
                # two windows' V matmuls column-tiled (array cols 0:63 /
                # 64:127) so both accumulate concurrently for the price
                # of one pass.  whi's data stays at partitions 64:128
                # through psum->sbuf evac (engine copies cannot shift
                # partitions); its transposes use the 64:128 diagonal of
                # the identity.
                box = {}

                def mk_mm(c, st, sp):
                    def f():
                        if st:
                            box["p"] = ps_vt.tile([128, W], f32, tag="vt", name="pvt2")
                        nc.tensor.matmul(
                            box["p"][0:64, :], wv_sb[:, c, :],
                            xt_tiles[wlo][:, c, :], start=st, stop=sp,
                        )
                        nc.tensor.matmul(
                            box["p"][64:128, :], wv_sb[:, c, :],
                            xt_tiles[whi][:, c, :], start=st, stop=sp,
                            tile_position=(0, 64),
                        )
                    return f

                def evac():
                    box["v"] = evac_pool.tile([128, W], pdt, tag="vtmp2", name="vt_pair")
                    nc.vector.tensor_copy(box["v"], box["p"])

                def mk_tr(w, half, t):
                    def f():
                        key = f"tr{half}"
                        if t == 0:
                            box[key] = ps_tr.tile([128, 4, H + 2], pdt, tag="tr", name=f"ptr{half}")
                        lo = 64 * half
                        nc.tensor.transpose(
                            box[key][:, t, 0:H],
                            box["v"][lo : lo + 64, t * 128 : (t + 1) * 128],
                            idp[lo : lo + 64, lo : lo + 64],
                        )
                    return f

                def mk_vcopy(w, half):
                    def f():
                        nc.vector.tensor_copy(
                            v_aug[:, 4 * w : 4 * w + 4, 0:H],
                            box[f"tr{half}"][:, :, 0:H],
                        )
                    return f

                items = [mk_mm(c, i == 0, i == NC_ - 1) for i, c in enumerate(range(NC_))]
                items.append(evac)
                for half, w in ((0, wlo), (1, whi)):
                    items += [mk_tr(w, half, t) for t in range(4)]
                    items.append(mk_vcopy(w, half))
                return items

            def out_evac(w, pouts):
                # emitted IMMEDIATELY at window close: frees the single
                # ps_out bank so the next window's PV WAR-dep is tracked
                box = {}
                box["oT"] = out_pool.tile([H + 1, W], f32, tag="oT", name="oT")
                nc.vector.tensor_copy(box["oT"], pouts.pop(w))
                return box

            def out_items(w, box):
                # unnormalized out^T straight to DRAM; host divides row 64
                def dma():
                    nc.sync.dma_start(out=Y[:, w, :], in_=box["oT"])

                return [dma]

            # ---------------- reservoir of interleavable work ----------
            reservoir = []  # list of (tag, closure)

            def add_group(tag, items):
                for it in items:
                    reservoir.append((tag, it))

            def add_group_front(tag, items):
                # K/Q projections gate the next window's attention start:
                # they must drain through fills before v/out leftovers
                for j, it in enumerate(items):
                    reservoir.insert(j, (tag, it))

            def deferred(tag):
                # V(3) is only consumed by window 3's PV stream; hold it
                # back so the exp-bound last window keeps PE fillers
                return tag == ("v", 3)

            def fill(n):
                for _ in range(n):
                    pick = None
                    for j, (tag, it) in enumerate(reservoir):
                        if not deferred(tag):
                            pick = j
                            break
                    if pick is None:
                        if not reservoir:
                            return
                        pick = 0
                    reservoir.pop(pick)[1]()

            def flush(pred):
                keep = []
                for tag, it in reservoir:
                    if pred(tag):
                        it()
                    else:
                        keep.append((tag, it))
                reservoir[:] = keep

            # ---------------- attention pair stream --------------------
            def lo_of(w, k):
                d = 128 * k - W * w
                return max(0, d) if TRIM else 0

            def emit_qk_exp(w, p, pts):
                # trimmed halves packed around the bank boundary: h0 at
                # [lo0:W] (end of psum bank 0), h1 at [W:W+n1] (start of
                # bank 1) — contiguous so ONE exp covers the pair
                # (activations carry ~260ns fixed overhead per instr)
                # while the two concurrent row-tiled matmuls never write
                # the same psum bank.
                win0 = w * W
                k0, k1 = 2 * p, 2 * p + 1
                paff = ps_aff.tile([128, 2 * W], f32, tag="aff")
                pt = pt_pool.tile([128, 2 * W], pdt, tag="pt")
                lo0, lo1 = lo_of(w, k0), lo_of(w, k1)
                n1 = W - lo1
                pts[(w, p)] = (pt, lo0, n1)
                # row-tiled pair: block k0 in PE rows 0..63,
                # block k1 concurrently in rows 64..127
                nc.tensor.matmul(
                    paff[:, lo0:W],
                    kq_all[0:64, k0 * 128 : (k0 + 1) * 128],
                    kqswap[0:64, win0 + lo0 : win0 + W],
                    start=True, stop=True,
                )
                nc.tensor.matmul(
                    paff[:, W : W + n1],
                    kqswap[64:128, k1 * 128 : (k1 + 1) * 128],
                    kq_all[64:128, win0 + lo1 : win0 + W],
                    start=True, stop=True,
                )
                nc.scalar.activation(
                    out=pt[:, lo0 : W + n1], in_=paff[:, lo0 : W + n1], func=Exp
                )
                if k1 >= 4 * w:  # pair contains (partially) masked blocks
                    for h, (k, lo, off) in ((0, (k0, lo0, 0)), (1, (k1, lo1, W))):
                        delta = 128 * k - W * w
                        if delta > -128:
                            lom = max(0, delta) if TRIM else 0
                            him = min(W, delta + 128) if TRIM else W
                            if h == 0:
                                half = pt[:, lom:him]
                                # zero where sk > sq via 0/1 mask multiply
                                nc.vector.tensor_mul(
                                    half, half,
                                    mask_sb[:, 384 - delta + lom : 384 - delta + him],
                                )
                            else:
                                half = pt[:, W + lom - lo : W + him - lo]
                                # same predicate on the gpsimd engine so
                                # the two halves mask in parallel
                                nc.gpsimd.affine_select(
                                    out=half, in_=half,
                                    compare_op=ge, fill=0.0,
                                    base=lom - delta,
                                    pattern=[[1, him - lom]],
                                    channel_multiplier=-1,
                                )

            def emit_pv(w, p, pts, pouts):
                if p == 0:
                    pouts[w] = ps_out.tile([H + 1, W], f32, tag="out", name="pout")
                pout = pouts[w]
                nblk = 4 * w + 4
                pt, lo0, n1 = pts.pop((w, p))
                for h, (k, lo, off) in (
                    (0, (2 * p, lo0, lo0)),
                    (1, (2 * p + 1, W - n1, W)),
                ):
                    nc.tensor.matmul(
                        pout[:, lo:W],
                        v_aug[:, k, :],
                        pt[:, off : off + (W - lo)],
                        start=(k == 0), stop=(k == nblk - 1),
                    )

            # ---------------- main schedule ----------------------------
            # proj(0) emitted directly, KQ/V chunk matmuls interleaved so
            # the PE tracks the DMA chunk-arrival pace without idling;
            # later windows' projections ride the reservoir.
            kq0 = kq_items(0)
            v0 = v_items(0)
            for c in range(NC_):
                kq0[c]()
                v0[c]()
            for it in kq0[NC_:]:   # kq evac + swap
                it()
            for it in v0[NC_:]:    # v evac + transposes + vcopy
                it()
            nxt = {0: 1, 1: 2, 2: 3, 3: None}  # filler proj after window
            fills = {0: 5, 1: 5, 2: 5, 3: 4}
            pts, pouts = {}, {}
            pairs = [(w, p) for w in W_ORDER for p in range(2 * w + 2)]
            emitted_proj = {0}
            oT3 = None
            for i, (w, p) in enumerate(pairs):
                if p == 0:
                    # barrier: this window's K/Q projection must be emitted
                    flush(lambda t: t == ("kq", w) or t == ("out",))
                emit_qk_exp(w, p, pts)
                if p == 0 and nxt[w] is not None and nxt[w] not in emitted_proj:
                    u = nxt[w]
                    emitted_proj.add(u)
                    add_group_front(("kq", u), kq_items(u))
                    if u == 2:
                        # V(1)+V(2) run column-tiled as one pass; tag as
                        # ("v", 1) so any flush needing V1 forces both
                        add_group(("v", 1), v_items_pair(1, 2))
                    elif u != 1:
                        add_group(("v", u), v_items(u))
                fill(fills[w])
                if i > 0:
                    wp, pp = pairs[i - 1]
                    # barrier: PV(wp,pp) reads v_aug blocks 2pp,2pp+1
                    vneed = (2 * pp + 1) // 4
                    flush(lambda t: t[0] == "v" and t[1] <= vneed)
                    emit_pv(wp, pp, pts, pouts)
                    if pp == 2 * wp + 1:  # closed window wp
                        box = out_evac(wp, pouts)
                        add_group(("out",), out_items(wp, box))
                    elif (wp, pp) == (3, 6):
                        # cols 0:256 of window 3's out^T are final after
                        # pair 6 (pair 7's diagonal blocks only touch
                        # 256:512): ship them before the last pair so the
                        # end-of-kernel DMA+receipt only covers half
                        oT3 = out_pool.tile([H + 1, W], f32, tag="oT", name="oT3")
                        nc.vector.tensor_copy(oT3[:, 0:256], pouts[3][:, 0:256])
                        nc.sync.dma_start(out=Y[:, 3, 0:256], in_=oT3[:, 0:256])
            wl, pl = pairs[-1]
            flush(lambda t: t[0] == "v" and t[1] <= (2 * pl + 1) // 4)
            emit_pv(wl, pl, pts, pouts)
            flush(lambda t: True)
            nc.vector.tensor_copy(oT3[:, 256:512], pouts.pop(wl)[:, 256:512])
            nc.sync.dma_start(out=Y[:, wl, 256:512], in_=oT3[:, 256:512])

    nc.finalize()
    return nc


def _np_dt(name):
    if name == "bf16":
        import ml_dtypes

        return ml_dtypes.bfloat16
    return np.float32


def _host_inputs(X, Wk, Wq, Wv):
    """Per-core input dicts (host-side sharding + layout prep)."""
    xnp = _np_dt(XDT_NAME)
    scale = 1.0 / np.sqrt(np.float32(D))
    wkq = np.concatenate([Wk, Wq * scale], axis=0).T  # [D, 128]
    wkq = np.ascontiguousarray(
        wkq.reshape(NC_, 128, 128).transpose(1, 0, 2)
    ).astype(xnp)  # [p, c, m]
    wv = np.ascontiguousarray(
        Wv.T.reshape(NC_, 128, H).transpose(1, 0, 2)
    ).astype(xnp)  # [p, c, h]
    pnp = _np_dt(PDT_NAME)
    idt = np.eye(128, dtype=np.float32).astype(pnp)

    in_maps = []
    for b in range(N_CORES):
        xt = np.ascontiguousarray(
            X[b].T.reshape(NC_, 128, NW, W).transpose(1, 2, 0, 3)
        ).astype(xnp)  # [p, w, c, s]
        in_maps.append({"XT": xt, "WKQ": wkq, "WV": wv, "IDT": idt})
    return in_maps


def _unshard(results):
    """Device Y is unnormalized out^T [h(+denom), w, s]; divide + transpose."""
    outs = []
    for i in range(N_CORES):
        y = results[i]["Y"]  # [65, NW, W] f32
        o = y[:H] / y[H : H + 1]  # [64, 4, 512]
        outs.append(np.ascontiguousarray(o.transpose(1, 2, 0)).reshape(S, H))
    return np.stack(outs, axis=0).astype(np.float32)


def kernel(X, Wk, Wq, Wv):
    global _compiled
    from concourse.bass_utils import run_bass_kernel_spmd

    if _compiled is None:
        _compiled = _build()
    in_maps = _host_inputs(
        np.asarray(X, dtype=np.float32),
        np.asarray(Wk, dtype=np.float32),
        np.asarray(Wq, dtype=np.float32),
        np.asarray(Wv, dtype=np.float32),
    )
    res = run_bass_kernel_spmd(_compiled, in_maps, list(range(N_CORES)))
    return _unshard(res.results)


# revision 35
# speedup vs baseline: 1.0580x; 1.0414x over previous
"""Trainium2 Bass kernel for a single-head causal attention block.

Reference computation (per batch element b):
    q = X[b] @ Wq.T ; k = X[b] @ Wk.T ; v = X[b] @ Wv.T        # [S, H]
    aff = softmax(causal_mask(q @ k.T / sqrt(D)))              # [S, S]
    out[b] = aff @ v                                           # [S, H]

Sharding: data-parallel over batch — 8 batch elements, 8 NeuronCores,
one batch element per core. Weights replicated.

Per-core layout strategy ("transposed attention"):
  - Host supplies X^T pre-tiled [p, w, c, s] so every DMA is contiguous.
  - K^T/Q^T are produced by one M=128 matmul pass (lhsT = [Wk; Wq*scale]
    chunks), V^T by an M=64 pass; V^T is PE-transposed into [sk, 64] tiles
    augmented with a ones column (col 64).
  - Attention works on aff^T blocks [sk=128, sq=512]: exp() on ScalarE,
    causal zeroing via DVE mask-mul / gpsimd affine_select, then
    out^T [65, 512] += V_aug[k].T @ P^T accumulated over sk blocks.
    Row 64 of out^T is the softmax denominator (ones column).
  - QK matmuls are row-tiled: even block of a pair runs in PE rows 0..63
    and the odd block concurrently in rows 64..127 (K^T/Q^T replicas
    parked at partitions 64..127 via a PE permutation matmul).
  - Causal trimming: diagonal blocks only compute/exp/mask/accumulate
    columns [delta:512]; fully masked blocks are never emitted.
  - out^T (numerator rows 0..63 + denominator row 64) is DMA'd to DRAM
    untransposed and unnormalized; the host does the divide + transpose.
  - Global software pipeline: all windows' attention pairs form one
    stream; the next windows' projection matmuls are injected between
    pairs ("fillers"). V(3) fillers are held back so the exp-bound final
    window still has PE work to overlap with ScalarE.
  - Startup: warmup matmuls from t=0 lift the PE HAM clock-gate while
    prioritized fine-grained DMAs (wkq, then xt0 per-2-chunk across all
    three queue engines) land the first window's data ~2us in.
"""

import sys

if "/opt/trn_rl_repo" not in sys.path:
    sys.path.insert(0, "/opt/trn_rl_repo")

import numpy as np

B, S, D, H = 8, 2048, 1024, 64
N_CORES = 8
W = 512           # sq window width
NW = S // W       # 4 windows
NC_ = D // 128    # 8 d-chunks
NB = S // 128     # 16 sk blocks

XDT_NAME = "bf16"   # X / projection weights / Q^T / K^T
PDT_NAME = "bf16"   # V tiles and exp(aff) (PV matmul operands)

N_WARMUP_MM = 6     # dense warmup matmuls at t=0 (HAM clock-gate lift)
WARM_N = 512        # warmup matmul moving width
W_ORDER = [0, 1, 2, 3]   # attention window processing order
FILL_PER_PAIR = 4        # reservoir items drained after each QK pair
TRIM = True              # causal column trimming on diagonal blocks

_compiled = None


def _build():
    import concourse.mybir as mybir
    import concourse.tile as tile
    from concourse import bacc

    f32 = mybir.dt.float32
    xdt = getattr(mybir.dt, {"bf16": "bfloat16", "f32r": "float32r"}[XDT_NAME])
    pdt = getattr(mybir.dt, {"bf16": "bfloat16", "f32r": "float32r"}[PDT_NAME])

    nc = bacc.Bacc(None, target_bir_lowering=False)

    XT = nc.declare_dram_parameter("XT", [128, NW, NC_, W], xdt, isOutput=False)
    WKQ = nc.declare_dram_parameter("WKQ", [128, NC_, 128], xdt, isOutput=False)
    WV = nc.declare_dram_parameter("WV", [128, NC_, H], xdt, isOutput=False)
    IDT = nc.declare_dram_parameter("IDT", [128, 128], pdt, isOutput=False)
    Y = nc.declare_dram_parameter("Y", [H + 1, NW, W], f32, isOutput=True)

    Exp = mybir.ActivationFunctionType.Exp
    ge = mybir.AluOpType.is_ge

    with tile.TileContext(nc) as tc:
        with (
            tc.tile_pool(name="const", bufs=1) as const,
            tc.tile_pool(name="persist", bufs=1) as persist,
            tc.tile_pool(name="evac", bufs=2) as evac_pool,
            tc.tile_pool(name="pt", bufs=4) as pt_pool,
            tc.tile_pool(name="outp", bufs=2) as out_pool,
            tc.tile_pool(name="ps_kq", bufs=1, space="PSUM") as ps_kq,
            tc.tile_pool(name="ps_vt", bufs=1, space="PSUM") as ps_vt,
            tc.tile_pool(name="ps_tr", bufs=1, space="PSUM") as ps_tr,
            tc.tile_pool(name="ps_aff", bufs=2, space="PSUM") as ps_aff,
            tc.tile_pool(name="ps_out", bufs=1, space="PSUM") as ps_out,
        ):
            # ---- PE warmup: dense matmuls from t=0 so the HAM clock-gate
            # opens (~3.4us sustained busy) while startup DMAs land ----
            scratch = const.tile([128, WARM_N], xdt)
            nc.vector.memset(scratch, 0.0)
            exp_warm = const.tile([128, 2], f32)
            nc.vector.memset(exp_warm[:, 0:1], 0.0)
            # wide warmups for bulk busy-time, then a short-matmul tail so
            # the handoff to the first (DMA-gated) projection matmul never
            # leaves a PE idle gap big enough to re-arm the HAM throttle
            # run the warmup PAST the point where xt0 is fully resident
            # (~14us): chasing individual chunk arrivals leaves PE gaps
            # whenever HBM receipt jitters, and one >1.7us gap re-arms
            # the HAM throttle for 3-7us.  A dependency-free bulk keeps
            # the busy-window unbroken; the projection then runs at
            # guaranteed-warm speed with every chunk already in SBUF.
            warm = ps_aff.tile([128, 2 * W], f32, tag="aff")
            warm_widths = [WARM_N] * 9 + [128] * 4
            for i, wn in enumerate(warm_widths):
                nc.tensor.matmul(
                    warm[:, 0:wn], scratch[:, 0:128], scratch[:, 0:wn],
                    start=(i == 0), stop=(i == len(warm_widths) - 1),
                )

            # ---- startup DMAs: each dma_start costs ~0.7us of issue time
            # on its engine and aggregate HBM is ~325 GB/s, so the three
            # queues carry ONLY wkq + xt0 (+ tiny wv/idp) until window 0
            # is resident; xt2/xt3 are gated behind xt0 so they cannot
            # steal bandwidth from the critical window ----
            wkq_sb = const.tile([128, NC_, 128], xdt)
            xt_tiles = []
            for w in range(NW):
                xt_tiles.append(persist.tile([128, NC_, W], xdt, tag=f"xt{w}", name=f"xt{w}"))
            wv_sb = const.tile([128, NC_, H], xdt)
            idp = const.tile([128, 128], pdt)

            # sync queue: first wkq chunks gate the very first matmul;
            # per-queue FIFO then serves xt0 tail -> xt1 -> xt2 -> xt3 in
            # exactly criticality order without stealing bandwidth early
            # NOTE: every dma_start costs its queue ~1us of completion
            # dead-time, so xt0 ships as just TWO big DMAs
            nc.sync.dma_start(out=wkq_sb[:, 0:2, :], in_=WKQ[:, 0:2, :])
            nc.sync.dma_start(out=wkq_sb[:, 2:8, :], in_=WKQ[:, 2:8, :])
            nc.sync.dma_start(out=xt_tiles[0][:, 4:8, :], in_=XT[:, 0, 4:8, :])
            nc.sync.dma_start(out=xt_tiles[1][:, 4:8, :], in_=XT[:, 1, 4:8, :])
            nc.sync.dma_start(out=xt_tiles[2][:, 0:4, :], in_=XT[:, 2, 0:4, :])
            nc.sync.dma_start(out=xt_tiles[2][:, 4:8, :], in_=XT[:, 2, 4:8, :])
            nc.sync.dma_start(out=xt_tiles[3][:, 0:4, :], in_=XT[:, 3, 0:4, :])
            nc.sync.dma_start(out=xt_tiles[3][:, 4:8, :], in_=XT[:, 3, 4:8, :])
            # scalar queue: xt0 low chunks (first matmul dependency)
            nc.scalar.dma_start(out=xt_tiles[0][:, 0:4, :], in_=XT[:, 0, 0:4, :])
            nc.scalar.dma_start(out=xt_tiles[1][:, 0:4, :], in_=XT[:, 1, 0:4, :])
            # gpsimd queue: the two small constants
            nc.gpsimd.dma_start(out=wv_sb, in_=WV[:, :, :])
            nc.gpsimd.dma_start(out=idp, in_=IDT[:, :])

            # preload the Exp activation table while startup DMAs run
            nc.scalar.activation(out=exp_warm[:, 1:2], in_=exp_warm[:, 0:1], func=Exp)

            # causal mask strip generated on device: M[p, c] = (c - p >= 384)
            mask_sb = const.tile([128, 896], pdt)
            nc.vector.memset(mask_sb, 1.0)
            nc.gpsimd.affine_select(
                out=mask_sb, in_=mask_sb,
                compare_op=ge, fill=0.0,
                base=-384, pattern=[[1, 896]], channel_multiplier=-1,
            )
            # swap permutation (64-rotated identity) built from idp
            swp_sb = const.tile([128, 128], xdt)
            nc.vector.tensor_copy(swp_sb[:, 0:64], idp[:, 64:128])
            nc.vector.tensor_copy(swp_sb[:, 64:128], idp[:, 0:64])


            # kq_all rows 0..63 = K^T, rows 64..127 = Q^T (as projected).
            # kqswap = 64-rotated copy (rows 0..63 = Q^T, rows 64..127 = K^T)
            # produced by a PE permutation matmul — SBUF-SBUF DMA shifts have
            # multi-us latency and serialized the whole attention stream.
            kq_all = persist.tile([128, S], xdt)
            kqswap = persist.tile([128, S], xdt)
            v_aug = persist.tile([128, NB, H + 1], pdt)  # V blocks + ones col
            ones_sb = const.tile([128, 1], f32)
            nc.vector.memset(ones_sb, 1.0)
            for k in range(NB):
                nc.vector.tensor_copy(v_aug[:, k, H : H + 1], ones_sb)

            # ---------------- projection emission (as filler items) ----
            def kq_items(w, order=None):
                win = slice(w * W, (w + 1) * W)
                xt_w = xt_tiles[w]
                order = order if order is not None else list(range(NC_))
                pkq_box = {}

                def mk_mm(c, st, sp):
                    def f():
                        if st:
                            pkq_box["t"] = ps_kq.tile([128, W], f32, tag="kq", name="pkq")
                        nc.tensor.matmul(
                            pkq_box["t"], wkq_sb[:, c, :], xt_w[:, c, :],
                            start=st, stop=sp,
                        )
                    return f

                def evac():
                    nc.vector.tensor_copy(kq_all[:, win], pkq_box["t"])

                def swap():
                    psw = ps_kq.tile([128, W], f32, tag="kq", name="psw")
                    nc.tensor.matmul(
                        psw, swp_sb, kq_all[:, win], start=True, stop=True
                    )
                    nc.vector.tensor_copy(kqswap[:, win], psw)

                return [mk_mm(c, i == 0, i == NC_ - 1)
                        for i, c in enumerate(order)] + [evac, swap]

            def v_items(w, order=None):
                xt_w = xt_tiles[w]
                order = order if order is not None else list(range(NC_))
                box = {}

                def mk_mm(c, st, sp):
                    def f():
                        if st:
                            box["p"] = ps_vt.tile([64, W], f32, tag="vt", name="pvt")
                        nc.tensor.matmul(
                            box["p"], wv_sb[:, c, :], xt_w[:, c, :],
                            start=st, stop=sp,
                        )
                    return f

                def evac():
                    box["v"] = evac_pool.tile([64, W], pdt, tag="vtmp", name="vt_tmp")
                    nc.vector.tensor_copy(box["v"], box["p"])

                def mk_tr(t):
                    def f():
                        if t == 0:
                            box["tr"] = ps_tr.tile([128, 4, H + 2], pdt, tag="tr", name="ptr")
                        nc.tensor.transpose(
                            box["tr"][:, t, 0:H],
                            box["v"][:, t * 128 : (t + 1) * 128],
                            idp[0:64, 0:64],
                        )
                    return f

                def vcopy():
                    nc.vector.tensor_copy(
                        v_aug[:, 4 * w : 4 * w + 4, 0:H], box["tr"][:, :, 0:H]
                    )

                return [mk_mm(c, i == 0, i == NC_ - 1)
                        for i, c in enumerate(order)] + [evac] \
                    + [mk_tr(t) for t in range(4)] + [vcopy]

            def v_items_pair(wlo, whi):
                # V projections have M=64 — half the PE array idle.  Run
                # two windows' V matmuls column-tiled (array cols 0:63 /
                # 64:127) so both accumulate concurrently for the price
                # of one pass.  whi's data stays at partitions 64:128
                # through psum->sbuf evac (engine copies cannot shift
                # partitions); its transposes use the 64:128 diagonal of
                # the identity.
                box = {}

                def mk_mm(c, st, sp):
                    def f():
                        if st:
                            box["p"] = ps_vt.tile([128, W], f32, tag="vt", name="pvt2")
                        nc.tensor.matmul(
                            box["p"][0:64, :], wv_sb[:, c, :],
                            xt_tiles[wlo][:, c, :], start=st, stop=sp,
                        )
                        nc.tensor.matmul(
                            box["p"][64:128, :], wv_sb[:, c, :],
                            xt_tiles[whi][:, c, :], start=st, stop=sp,
                            tile_position=(0, 64),
                        )
                    return f

                def evac():
                    box["v"] = evac_pool.tile([128, W], pdt, tag="vtmp2", name="vt_pair")
                    nc.vector.tensor_copy(box["v"], box["p"])

                def mk_tr(w, half, t):
                    def f():
                        key = f"tr{half}"
                        if t == 0:
                            box[key] = ps_tr.tile([128, 4, H + 2], pdt, tag="tr", name=f"ptr{half}")
                        lo = 64 * half
                        nc.tensor.transpose(
                            box[key][:, t, 0:H],
                            box["v"][lo : lo + 64, t * 128 : (t + 1) * 128],
                            idp[lo : lo + 64, lo : lo + 64],
                        )
                    return f

                def mk_vcopy(w, half):
                    def f():
                        nc.vector.tensor_copy(
                            v_aug[:, 4 * w : 4 * w + 4, 0:H],
                            box[f"tr{half}"][:, :, 0:H],
                        )
                    return f

                items = [mk_mm(c, i == 0, i == NC_ - 1) for i, c in enumerate(range(NC_))]
                items.append(evac)
                for half, w in ((0, wlo), (1, whi)):
                    items += [mk_tr(w, half, t) for t in range(4)]
                    items.append(mk_vcopy(w, half))
                return items

            def out_evac(w, pouts):
                # emitted IMMEDIATELY at window close: frees the single
                # ps_out bank so the next window's PV WAR-dep is tracked
                box = {}
                box["oT"] = out_pool.tile([H + 1, W], f32, tag="oT", name="oT")
                nc.vector.tensor_copy(box["oT"], pouts.pop(w))
                return box

            def out_items(w, box):
                # unnormalized out^T straight to DRAM; host divides row 64
                def dma():
                    nc.sync.dma_start(out=Y[:, w, :], in_=box["oT"])

                return [dma]

            # ---------------- reservoir of interleavable work ----------
            reservoir = []  # list of (tag, closure)

            def add_group(tag, items):
                for it in items:
                    reservoir.append((tag, it))

            def add_group_front(tag, items):
                # K/Q projections gate the next window's attention start:
                # they must drain through fills before v/out leftovers
                for j, it in enumerate(items):
                    reservoir.insert(j, (tag, it))

            def deferred(tag):
                # V(3) is only consumed by window 3's PV stream; hold it
                # back so the exp-bound last window keeps PE fillers
                return tag == ("v", 3)

            def fill(n):
                for _ in range(n):
                    pick = None
                    for j, (tag, it) in enumerate(reservoir):
                        if not deferred(tag):
                            pick = j
                            break
                    if pick is None:
                        if not reservoir:
                            return
                        pick = 0
                    reservoir.pop(pick)[1]()

            def flush(pred):
                keep = []
                for tag, it in reservoir:
                    if pred(tag):
                        it()
                    else:
                        keep.append((tag, it))
                reservoir[:] = keep

            # ---------------- attention pair stream --------------------
            def lo_of(w, k):
                d = 128 * k - W * w
                return max(0, d) if TRIM else 0

            def emit_qk_exp(w, p, pts):
                # trimmed halves packed around the bank boundary: h0 at
                # [lo0:W] (end of psum bank 0), h1 at [W:W+n1] (start of
                # bank 1) — contiguous so ONE exp covers the pair
                # (activations carry ~260ns fixed overhead per instr)
                # while the two concurrent row-tiled matmuls never write
                # the same psum bank.
                win0 = w * W
                k0, k1 = 2 * p, 2 * p + 1
                paff = ps_aff.tile([128, 2 * W], f32, tag="aff")
                pt = pt_pool.tile([128, 2 * W], pdt, tag="pt")
                lo0, lo1 = lo_of(w, k0), lo_of(w, k1)
                n1 = W - lo1
                pts[(w, p)] = (pt, lo0, n1)
                # row-tiled pair: block k0 in PE rows 0..63,
                # block k1 concurrently in rows 64..127
                nc.tensor.matmul(
                    paff[:, lo0:W],
                    kq_all[0:64, k0 * 128 : (k0 + 1) * 128],
                    kqswap[0:64, win0 + lo0 : win0 + W],
                    start=True, stop=True,
                )
                nc.tensor.matmul(
                    paff[:, W : W + n1],
                    kqswap[64:128, k1 * 128 : (k1 + 1) * 128],
                    kq_all[64:128, win0 + lo1 : win0 + W],
                    start=True, stop=True,
                )
                nc.scalar.activation(
                    out=pt[:, lo0 : W + n1], in_=paff[:, lo0 : W + n1], func=Exp
                )
                if k1 >= 4 * w:  # pair contains (partially) masked blocks
                    for h, (k, lo, off) in ((0, (k0, lo0, 0)), (1, (k1, lo1, W))):
                        delta = 128 * k - W * w
                        if delta > -128:
                            lom = max(0, delta) if TRIM else 0
                            him = min(W, delta + 128) if TRIM else W
                            if h == 0:
                                half = pt[:, lom:him]
                                # zero where sk > sq via 0/1 mask multiply
                                nc.vector.tensor_mul(
                                    half, half,
                                    mask_sb[:, 384 - delta + lom : 384 - delta + him],
                                )
                            else:
                                half = pt[:, W + lom - lo : W + him - lo]
                                # same predicate on the gpsimd engine so
                                # the two halves mask in parallel
                                nc.gpsimd.affine_select(
                                    out=half, in_=half,
                                    compare_op=ge, fill=0.0,
                                    base=lom - delta,
                                    pattern=[[1, him - lom]],
                                    channel_multiplier=-1,
                                )

            def emit_pv(w, p, pts, pouts):
                if p == 0:
                    pouts[w] = ps_out.tile([H + 1, W], f32, tag="out", name="pout")
                pout = pouts[w]
                nblk = 4 * w + 4
                pt, lo0, n1 = pts.pop((w, p))
                for h, (k, lo, off) in (
                    (0, (2 * p, lo0, lo0)),
                    (1, (2 * p + 1, W - n1, W)),
                ):
                    nc.tensor.matmul(
                        pout[:, lo:W],
                        v_aug[:, k, :],
                        pt[:, off : off + (W - lo)],
                        start=(k == 0), stop=(k == nblk - 1),
                    )

            # ---------------- main schedule ----------------------------
            # proj(0) emitted directly, KQ/V chunk matmuls interleaved so
            # the PE tracks the DMA chunk-arrival pace without idling;
            # later windows' projections ride the reservoir.
            kq0 = kq_items(0)
            v0 = v_items(0)
            for c in range(NC_):
                kq0[c]()
                v0[c]()
            for it in kq0[NC_:]:   # kq evac + swap
                it()
            for it in v0[NC_:]:    # v evac + transposes + vcopy
                it()
            nxt = {0: 1, 1: 2, 2: 3, 3: None}  # filler proj after window
            fills = {0: 5, 1: 5, 2: 5, 3: 4}
            pts, pouts = {}, {}
            pairs = [(w, p) for w in W_ORDER for p in range(2 * w + 2)]
            emitted_proj = {0}
            oT3 = None
            for i, (w, p) in enumerate(pairs):
                if p == 0:
                    # barrier: this window's K/Q projection must be emitted
                    flush(lambda t: t == ("kq", w) or t == ("out",))
                emit_qk_exp(w, p, pts)
                if p == 0 and nxt[w] is not None and nxt[w] not in emitted_proj:
                    u = nxt[w]
                    emitted_proj.add(u)
                    add_group_front(("kq", u), kq_items(u))
                    if u == 2:
                        # V(1)+V(2) run column-tiled as one pass; tag as
                        # ("v", 1) so any flush needing V1 forces both
                        add_group(("v", 1), v_items_pair(1, 2))
                    elif u != 1:
                        add_group(("v", u), v_items(u))
                fill(fills[w])
                if i > 0:
                    wp, pp = pairs[i - 1]
                    # barrier: PV(wp,pp) reads v_aug blocks 2pp,2pp+1
                    vneed = (2 * pp + 1) // 4
                    flush(lambda t: t[0] == "v" and t[1] <= vneed)
                    emit_pv(wp, pp, pts, pouts)
                    if pp == 2 * wp + 1:  # closed window wp
                        box = out_evac(wp, pouts)
                        add_group(("out",), out_items(wp, box))
                    elif (wp, pp) == (3, 6):
                        # cols 0:256 of window 3's out^T are final after
                        # pair 6 (pair 7's diagonal blocks only touch
                        # 256:512): ship them before the last pair so the
                        # end-of-kernel DMA+receipt only covers half
                        oT3 = out_pool.tile([H + 1, W], f32, tag="oT", name="oT3")
                        nc.vector.tensor_copy(oT3[:, 0:256], pouts[3][:, 0:256])
                        nc.sync.dma_start(out=Y[:, 3, 0:256], in_=oT3[:, 0:256])
            wl, pl = pairs[-1]
            flush(lambda t: t[0] == "v" and t[1] <= (2 * pl + 1) // 4)
            emit_pv(wl, pl, pts, pouts)
            flush(lambda t: True)
            nc.vector.tensor_copy(oT3[:, 256:512], pouts.pop(wl)[:, 256:512])
            nc.sync.dma_start(out=Y[:, wl, 256:512], in_=oT3[:, 256:512])

    nc.finalize()
    return nc


def _np_dt(name):
    if name == "bf16":
        import ml_dtypes

        return ml_dtypes.bfloat16
    return np.float32


def _host_inputs(X, Wk, Wq, Wv):
    """Per-core input dicts (host-side sharding + layout prep)."""
    xnp = _np_dt(XDT_NAME)
    scale = 1.0 / np.sqrt(np.float32(D))
    wkq = np.concatenate([Wk, Wq * scale], axis=0).T  # [D, 128]
    wkq = np.ascontiguousarray(
        wkq.reshape(NC_, 128, 128).transpose(1, 0, 2)
    ).astype(xnp)  # [p, c, m]
    wv = np.ascontiguousarray(
        Wv.T.reshape(NC_, 128, H).transpose(1, 0, 2)
    ).astype(xnp)  # [p, c, h]
    pnp = _np_dt(PDT_NAME)
    idt = np.eye(128, dtype=np.float32).astype(pnp)

    in_maps = []
    for b in range(N_CORES):
        xt = np.ascontiguousarray(
            X[b].T.reshape(NC_, 128, NW, W).transpose(1, 2, 0, 3)
        ).astype(xnp)  # [p, w, c, s]
        in_maps.append({"XT": xt, "WKQ": wkq, "WV": wv, "IDT": idt})
    return in_maps


def _unshard(results):
    """Device Y is unnormalized out^T [h(+denom), w, s]; divide + transpose."""
    outs = []
    for i in range(N_CORES):
        y = results[i]["Y"]  # [65, NW, W] f32
        o = y[:H] / y[H : H + 1]  # [64, 4, 512]
        outs.append(np.ascontiguousarray(o.transpose(1, 2, 0)).reshape(S, H))
    return np.stack(outs, axis=0).astype(np.float32)


def kernel(X, Wk, Wq, Wv):
    global _compiled
    from concourse.bass_utils import run_bass_kernel_spmd

    if _compiled is None:
        _compiled = _build()
    in_maps = _host_inputs(
        np.asarray(X, dtype=np.float32),
        np.asarray(Wk, dtype=np.float32),
        np.asarray(Wq, dtype=np.float32),
        np.asarray(Wv, dtype=np.float32),
    )
    res = run_bass_kernel_spmd(_compiled, in_maps, list(range(N_CORES)))
    return _unshard(res.results)


# revision 40
# speedup vs baseline: 1.0802x; 1.0210x over previous
"""Trainium2 Bass kernel for a single-head causal attention block.

Reference computation (per batch element b):
    q = X[b] @ Wq.T ; k = X[b] @ Wk.T ; v = X[b] @ Wv.T        # [S, H]
    aff = softmax(causal_mask(q @ k.T / sqrt(D)))              # [S, S]
    out[b] = aff @ v                                           # [S, H]

Sharding: data-parallel over batch — 8 batch elements, 8 NeuronCores,
one batch element per core. Weights replicated.

Per-core layout strategy ("transposed attention"):
  - Host supplies X^T pre-tiled [p, w, c, s] so every DMA is contiguous.
  - K^T/Q^T are produced by one M=128 matmul pass (lhsT = [Wk; Wq*scale]
    chunks), V^T by an M=64 pass; V^T is PE-transposed into [sk, 64] tiles
    augmented with a ones column (col 64).
  - Attention works on aff^T blocks [sk=128, sq=512]: exp() on ScalarE,
    causal zeroing via DVE mask-mul / gpsimd affine_select, then
    out^T [65, 512] += V_aug[k].T @ P^T accumulated over sk blocks.
    Row 64 of out^T is the softmax denominator (ones column).
  - QK matmuls are row-tiled: even block of a pair runs in PE rows 0..63
    and the odd block concurrently in rows 64..127 (K^T/Q^T replicas
    parked at partitions 64..127 via a PE permutation matmul).
  - Causal trimming: diagonal blocks only compute/exp/mask/accumulate
    columns [delta:512]; fully masked blocks are never emitted.
  - out^T (numerator rows 0..63 + denominator row 64) is DMA'd to DRAM
    untransposed and unnormalized; the host does the divide + transpose.
  - Global software pipeline: all windows' attention pairs form one
    stream; the next windows' projection matmuls are injected between
    pairs ("fillers"). V(3) fillers are held back so the exp-bound final
    window still has PE work to overlap with ScalarE.
  - Startup: warmup matmuls from t=0 lift the PE HAM clock-gate while
    prioritized fine-grained DMAs (wkq, then xt0 per-2-chunk across all
    three queue engines) land the first window's data ~2us in.
"""

import sys

if "/opt/trn_rl_repo" not in sys.path:
    sys.path.insert(0, "/opt/trn_rl_repo")

import numpy as np

B, S, D, H = 8, 2048, 1024, 64
N_CORES = 8
W = 512           # sq window width
NW = S // W       # 4 windows
NC_ = D // 128    # 8 d-chunks
NB = S // 128     # 16 sk blocks

XDT_NAME = "bf16"   # X / projection weights / Q^T / K^T
PDT_NAME = "bf16"   # V tiles and exp(aff) (PV matmul operands)

N_WARMUP_MM = 6     # dense warmup matmuls at t=0 (HAM clock-gate lift)
WARM_N = 512        # warmup matmul moving width
W_ORDER = [0, 1, 2, 3]   # attention window processing order
FILL_PER_PAIR = 4        # reservoir items drained after each QK pair
TRIM = True              # causal column trimming on diagonal blocks

_compiled = None


def _build():
    import concourse.mybir as mybir
    import concourse.tile as tile
    from concourse import bacc

    f32 = mybir.dt.float32
    xdt = getattr(mybir.dt, {"bf16": "bfloat16", "f32r": "float32r"}[XDT_NAME])
    pdt = getattr(mybir.dt, {"bf16": "bfloat16", "f32r": "float32r"}[PDT_NAME])

    nc = bacc.Bacc(None, target_bir_lowering=False)

    XT = nc.declare_dram_parameter("XT", [128, NW, NC_, W], xdt, isOutput=False)
    WKQ = nc.declare_dram_parameter("WKQ", [128, NC_, 128], xdt, isOutput=False)
    WV = nc.declare_dram_parameter("WV", [128, NC_, H], xdt, isOutput=False)
    IDT = nc.declare_dram_parameter("IDT", [128, 128], pdt, isOutput=False)
    Y = nc.declare_dram_parameter("Y", [H + 1, NW, W], f32, isOutput=True)

    Exp = mybir.ActivationFunctionType.Exp
    ge = mybir.AluOpType.is_ge

    with tile.TileContext(nc) as tc:
        with (
            tc.tile_pool(name="const", bufs=1) as const,
            tc.tile_pool(name="persist", bufs=1) as persist,
            tc.tile_pool(name="evac", bufs=2) as evac_pool,
            tc.tile_pool(name="pt", bufs=4) as pt_pool,
            tc.tile_pool(name="outp", bufs=2) as out_pool,
            tc.tile_pool(name="ps_kq", bufs=1, space="PSUM") as ps_kq,
            tc.tile_pool(name="ps_vt", bufs=1, space="PSUM") as ps_vt,
            tc.tile_pool(name="ps_tr", bufs=1, space="PSUM") as ps_tr,
            tc.tile_pool(name="ps_aff", bufs=2, space="PSUM") as ps_aff,
            tc.tile_pool(name="ps_out", bufs=1, space="PSUM") as ps_out,
        ):
            # ---- PE warmup: dense matmuls from t=0 so the HAM clock-gate
            # opens (~3.4us sustained busy) while startup DMAs land ----
            scratch = const.tile([128, WARM_N], xdt)
            nc.vector.memset(scratch, 0.0)
            exp_warm = const.tile([128, 2], f32)
            nc.vector.memset(exp_warm[:, 0:1], 0.0)
            # wide warmups for bulk busy-time, then a short-matmul tail so
            # the handoff to the first (DMA-gated) projection matmul never
            # leaves a PE idle gap big enough to re-arm the HAM throttle
            # run the warmup PAST the point where xt0 is fully resident
            # (~14us): chasing individual chunk arrivals leaves PE gaps
            # whenever HBM receipt jitters, and one >1.7us gap re-arms
            # the HAM throttle for 3-7us.  A dependency-free bulk keeps
            # the busy-window unbroken; the projection then runs at
            # guaranteed-warm speed with every chunk already in SBUF.
            warm = ps_aff.tile([128, 2 * W], f32, tag="aff")
            warm_widths = [WARM_N] * N_WARMUP_MM + [128] * 4
            for i, wn in enumerate(warm_widths):
                nc.tensor.matmul(
                    warm[:, 0:wn], scratch[:, 0:128], scratch[:, 0:wn],
                    start=(i == 0), stop=(i == len(warm_widths) - 1),
                )

            # ---- startup DMAs: each dma_start costs ~0.7us of issue time
            # on its engine and aggregate HBM is ~325 GB/s, so the three
            # queues carry ONLY wkq + xt0 (+ tiny wv/idp) until window 0
            # is resident; xt2/xt3 are gated behind xt0 so they cannot
            # steal bandwidth from the critical window ----
            wkq_sb = const.tile([128, NC_, 128], xdt)
            xt_tiles = []
            for w in range(NW):
                xt_tiles.append(persist.tile([128, NC_, W], xdt, tag=f"xt{w}", name=f"xt{w}"))
            wv_sb = const.tile([128, NC_, H], xdt)
            idp = const.tile([128, 128], pdt)

            # sync queue: first wkq chunks gate the very first matmul;
            # per-queue FIFO then serves xt0 tail -> xt1 -> xt2 -> xt3 in
            # exactly criticality order without stealing bandwidth early
            # NOTE: every dma_start costs its queue ~1us of completion
            # dead-time, so xt0 ships as just TWO big DMAs
            nc.sync.dma_start(out=wkq_sb[:, 0:2, :], in_=WKQ[:, 0:2, :])
            nc.sync.dma_start(out=wkq_sb[:, 2:8, :], in_=WKQ[:, 2:8, :])
            nc.sync.dma_start(out=xt_tiles[0][:, 6:8, :], in_=XT[:, 0, 6:8, :])
            nc.sync.dma_start(out=xt_tiles[0][:, 5:6, :], in_=XT[:, 0, 5:6, :])
            nc.sync.dma_start(out=xt_tiles[1][:, 4:8, :], in_=XT[:, 1, 4:8, :])
            nc.sync.dma_start(out=xt_tiles[2][:, 0:4, :], in_=XT[:, 2, 0:4, :])
            nc.sync.dma_start(out=xt_tiles[2][:, 4:8, :], in_=XT[:, 2, 4:8, :])
            nc.sync.dma_start(out=xt_tiles[3][:, 0:4, :], in_=XT[:, 3, 0:4, :])
            nc.sync.dma_start(out=xt_tiles[3][:, 4:8, :], in_=XT[:, 3, 4:8, :])
            # scalar queue: xt0 low chunks (first matmul dependency)
            nc.scalar.dma_start(out=xt_tiles[0][:, 0:2, :], in_=XT[:, 0, 0:2, :])
            nc.scalar.dma_start(out=xt_tiles[0][:, 2:4, :], in_=XT[:, 0, 2:4, :])
            nc.scalar.dma_start(out=xt_tiles[0][:, 4:5, :], in_=XT[:, 0, 4:5, :])
            nc.scalar.dma_start(out=xt_tiles[1][:, 0:4, :], in_=XT[:, 1, 0:4, :])
            # gpsimd queue: the two small constants
            nc.gpsimd.dma_start(out=wv_sb, in_=WV[:, :, :])
            nc.gpsimd.dma_start(out=idp, in_=IDT[:, :])

            # preload the Exp activation table while startup DMAs run
            nc.scalar.activation(out=exp_warm[:, 1:2], in_=exp_warm[:, 0:1], func=Exp)

            # causal mask strip generated on device: M[p, c] = (c - p >= 384)
            mask_sb = const.tile([128, 896], pdt)
            nc.vector.memset(mask_sb, 1.0)
            nc.gpsimd.affine_select(
                out=mask_sb, in_=mask_sb,
                compare_op=ge, fill=0.0,
                base=-384, pattern=[[1, 896]], channel_multiplier=-1,
            )
            # swap permutation (64-rotated identity) built from idp
            swp_sb = const.tile([128, 128], xdt)
            nc.vector.tensor_copy(swp_sb[:, 0:64], idp[:, 64:128])
            nc.vector.tensor_copy(swp_sb[:, 64:128], idp[:, 0:64])


            # kq_all rows 0..63 = K^T, rows 64..127 = Q^T (as projected).
            # kqswap = 64-rotated copy (rows 0..63 = Q^T, rows 64..127 = K^T)
            # produced by a PE permutation matmul — SBUF-SBUF DMA shifts have
            # multi-us latency and serialized the whole attention stream.
            kq_all = persist.tile([128, S], xdt)
            kqswap = persist.tile([128, S], xdt)
            v_aug = persist.tile([128, NB, H + 1], pdt)  # V blocks + ones col
            ones_sb = const.tile([128, 1], f32)
            nc.vector.memset(ones_sb, 1.0)
            for k in range(NB):
                nc.vector.tensor_copy(v_aug[:, k, H : H + 1], ones_sb)

            # ---------------- projection emission (as filler items) ----
            def kq_items(w, order=None):
                win = slice(w * W, (w + 1) * W)
                xt_w = xt_tiles[w]
                order = order if order is not None else list(range(NC_))
                pkq_box = {}

                def mk_mm(c, st, sp):
                    def f():
                        if st:
                            pkq_box["t"] = ps_kq.tile([128, W], f32, tag="kq", name="pkq")
                        nc.tensor.matmul(
                            pkq_box["t"], wkq_sb[:, c, :], xt_w[:, c, :],
                            start=st, stop=sp,
                        )
                    return f

                def evac():
                    nc.vector.tensor_copy(kq_all[:, win], pkq_box["t"])

                def swap():
                    psw = ps_kq.tile([128, W], f32, tag="kq", name="psw")
                    nc.tensor.matmul(
                        psw, swp_sb, kq_all[:, win], start=True, stop=True
                    )
                    nc.vector.tensor_copy(kqswap[:, win], psw)

                return [mk_mm(c, i == 0, i == NC_ - 1)
                        for i, c in enumerate(order)] + [evac, swap]

            def v_items(w, order=None):
                xt_w = xt_tiles[w]
                order = order if order is not None else list(range(NC_))
                box = {}

                def mk_mm(c, st, sp):
                    def f():
                        if st:
                            box["p"] = ps_vt.tile([64, W], f32, tag="vt", name="pvt")
                        nc.tensor.matmul(
                            box["p"], wv_sb[:, c, :], xt_w[:, c, :],
                            start=st, stop=sp,
                        )
                    return f

                def evac():
                    box["v"] = evac_pool.tile([64, W], pdt, tag="vtmp", name="vt_tmp")
                    nc.vector.tensor_copy(box["v"], box["p"])

                def mk_tr(t):
                    def f():
                        if t == 0:
                            box["tr"] = ps_tr.tile([128, 4, H + 2], pdt, tag="tr", name="ptr")
                        nc.tensor.transpose(
                            box["tr"][:, t, 0:H],
                            box["v"][:, t * 128 : (t + 1) * 128],
                            idp[0:64, 0:64],
                        )
                    return f

                def vcopy():
                    nc.vector.tensor_copy(
                        v_aug[:, 4 * w : 4 * w + 4, 0:H], box["tr"][:, :, 0:H]
                    )

                return [mk_mm(c, i == 0, i == NC_ - 1)
                        for i, c in enumerate(order)] + [evac] \
                    + [mk_tr(t) for t in range(4)] + [vcopy]

            def v_items_pair(wlo, whi):
                # V projections have M=64 — half the PE array idle.  Run
                # two windows' V matmuls column-tiled (array cols 0:63 /
                # 64:127) so both accumulate concurrently for the price
                # of one pass.  whi's data stays at partitions 64:128
                # through psum->sbuf evac (engine copies cannot shift
                # partitions); its transposes use the 64:128 diagonal of
                # the identity.
                box = {}

                def mk_mm(c, st, sp):
                    def f():
                        if st:
                            box["p"] = ps_vt.tile([128, W], f32, tag="vt", name="pvt2")
                        nc.tensor.matmul(
                            box["p"][0:64, :], wv_sb[:, c, :],
                            xt_tiles[wlo][:, c, :], start=st, stop=sp,
                        )
                        nc.tensor.matmul(
                            box["p"][64:128, :], wv_sb[:, c, :],
                            xt_tiles[whi][:, c, :], start=st, stop=sp,
                            tile_position=(0, 64),
                        )
                    return f

                def evac():
                    box["v"] = evac_pool.tile([128, W], pdt, tag="vtmp2", name="vt_pair")
                    nc.vector.tensor_copy(box["v"], box["p"])

                def mk_tr(w, half, t):
                    def f():
                        key = f"tr{half}"
                        if t == 0:
                            box[key] = ps_tr.tile([128, 4, H + 2], pdt, tag="tr", name=f"ptr{half}")
                        lo = 64 * half
                        nc.tensor.transpose(
                            box[key][:, t, 0:H],
                            box["v"][lo : lo + 64, t * 128 : (t + 1) * 128],
                            idp[lo : lo + 64, lo : lo + 64],
                        )
                    return f

                def mk_vcopy(w, half):
                    def f():
                        nc.vector.tensor_copy(
                            v_aug[:, 4 * w : 4 * w + 4, 0:H],
                            box[f"tr{half}"][:, :, 0:H],
                        )
                    return f

                items = [mk_mm(c, i == 0, i == NC_ - 1) for i, c in enumerate(range(NC_))]
                items.append(evac)
                for half, w in ((0, wlo), (1, whi)):
                    items += [mk_tr(w, half, t) for t in range(4)]
                    items.append(mk_vcopy(w, half))
                return items

            def out_evac(w, pouts):
                # emitted IMMEDIATELY at window close: frees the single
                # ps_out bank so the next window's PV WAR-dep is tracked
                box = {}
                box["oT"] = out_pool.tile([H + 1, W], f32, tag="oT", name="oT")
                nc.vector.tensor_copy(box["oT"], pouts.pop(w))
                return box

            def out_items(w, box):
                # unnormalized out^T straight to DRAM; host divides row 64
                def dma():
                    nc.sync.dma_start(out=Y[:, w, :], in_=box["oT"])

                return [dma]

            # ---------------- reservoir of interleavable work ----------
            reservoir = []  # list of (tag, closure)

            def add_group(tag, items):
                for it in items:
                    reservoir.append((tag, it))

            def add_group_front(tag, items):
                # K/Q projections gate the next window's attention start:
                # they must drain through fills before v/out leftovers
                for j, it in enumerate(items):
                    reservoir.insert(j, (tag, it))

            def deferred(tag):
                # V(3) is only consumed by window 3's PV stream; hold it
                # back so the exp-bound last window keeps PE fillers
                return tag == ("v", 3)

            def fill(n):
                for _ in range(n):
                    pick = None
                    for j, (tag, it) in enumerate(reservoir):
                        if not deferred(tag):
                            pick = j
                            break
                    if pick is None:
                        if not reservoir:
                            return
                        pick = 0
                    reservoir.pop(pick)[1]()

            def flush(pred):
                keep = []
                for tag, it in reservoir:
                    if pred(tag):
                        it()
                    else:
                        keep.append((tag, it))
                reservoir[:] = keep

            # ---------------- attention pair stream --------------------
            def lo_of(w, k):
                d = 128 * k - W * w
                return max(0, d) if TRIM else 0

            def emit_qk_exp(w, p, pts):
                # trimmed halves packed around the bank boundary: h0 at
                # [lo0:W] (end of psum bank 0), h1 at [W:W+n1] (start of
                # bank 1) — contiguous so ONE exp covers the pair
                # (activations carry ~260ns fixed overhead per instr)
                # while the two concurrent row-tiled matmuls never write
                # the same psum bank.
                win0 = w * W
                k0, k1 = 2 * p, 2 * p + 1
                paff = ps_aff.tile([128, 2 * W], f32, tag="aff")
                pt = pt_pool.tile([128, 2 * W], pdt, tag="pt")
                lo0, lo1 = lo_of(w, k0), lo_of(w, k1)
                n1 = W - lo1
                pts[(w, p)] = (pt, lo0, n1)
                # row-tiled pair: block k0 in PE rows 0..63,
                # block k1 concurrently in rows 64..127
                nc.tensor.matmul(
                    paff[:, lo0:W],
                    kq_all[0:64, k0 * 128 : (k0 + 1) * 128],
                    kqswap[0:64, win0 + lo0 : win0 + W],
                    start=True, stop=True,
                )
                nc.tensor.matmul(
                    paff[:, W : W + n1],
                    kqswap[64:128, k1 * 128 : (k1 + 1) * 128],
                    kq_all[64:128, win0 + lo1 : win0 + W],
                    start=True, stop=True,
                )
                nc.scalar.activation(
                    out=pt[:, lo0 : W + n1], in_=paff[:, lo0 : W + n1], func=Exp
                )
                if k1 >= 4 * w:  # pair contains (partially) masked blocks
                    for h, (k, lo, off) in ((0, (k0, lo0, 0)), (1, (k1, lo1, W))):
                        delta = 128 * k - W * w
                        if delta > -128:
                            lom = max(0, delta) if TRIM else 0
                            him = min(W, delta + 128) if TRIM else W
                            if h == 0:
                                half = pt[:, lom:him]
                                # zero where sk > sq via 0/1 mask multiply
                                nc.vector.tensor_mul(
                                    half, half,
                                    mask_sb[:, 384 - delta + lom : 384 - delta + him],
                                )
                            else:
                                half = pt[:, W + lom - lo : W + him - lo]
                                # same predicate on the gpsimd engine so
                                # the two halves mask in parallel
                                nc.gpsimd.affine_select(
                                    out=half, in_=half,
                                    compare_op=ge, fill=0.0,
                                    base=lom - delta,
                                    pattern=[[1, him - lom]],
                                    channel_multiplier=-1,
                                )

            def emit_pv(w, p, pts, pouts):
                if p == 0:
                    pouts[w] = ps_out.tile([H + 1, W], f32, tag="out", name="pout")
                pout = pouts[w]
                nblk = 4 * w + 4
                pt, lo0, n1 = pts.pop((w, p))
                for h, (k, lo, off) in (
                    (0, (2 * p, lo0, lo0)),
                    (1, (2 * p + 1, W - n1, W)),
                ):
                    nc.tensor.matmul(
                        pout[:, lo:W],
                        v_aug[:, k, :],
                        pt[:, off : off + (W - lo)],
                        start=(k == 0), stop=(k == nblk - 1),
                    )

            # ---------------- main schedule ----------------------------
            # proj(0) emitted directly, KQ/V chunk matmuls interleaved so
            # the PE tracks the DMA chunk-arrival pace without idling;
            # later windows' projections ride the reservoir.
            kq0 = kq_items(0)
            v0 = v_items(0)
            for c in range(NC_):
                if c == 4:
                    # c4/c5 arrive a shade behind the PE's warm pace;
                    # two short stuffer matmuls bridge the supply seam
                    st = ps_aff.tile([128, 2 * W], f32, tag="aff")
                    nc.tensor.matmul(st[:, 0:256], scratch[:, 0:128],
                                     scratch[:, 0:256], start=True, stop=False)
                    nc.tensor.matmul(st[:, 0:256], scratch[:, 0:128],
                                     scratch[:, 0:256], start=False, stop=True)
                kq0[c]()
                v0[c]()
            for it in kq0[NC_:]:   # kq evac + swap
                it()
            for it in v0[NC_:]:    # v evac + transposes + vcopy
                it()
            nxt = {0: 1, 1: 2, 2: 3, 3: None}  # filler proj after window
            fills = {0: 4, 1: 4, 2: 4, 3: 4}
            pts, pouts = {}, {}
            pairs = [(w, p) for w in W_ORDER for p in range(2 * w + 2)]
            emitted_proj = {0}
            oT3 = None
            for i, (w, p) in enumerate(pairs):
                if p == 0:
                    # barrier: this window's K/Q projection must be emitted
                    flush(lambda t: t == ("kq", w) or t == ("out",))
                emit_qk_exp(w, p, pts)
                if p == 0 and nxt[w] is not None and nxt[w] not in emitted_proj:
                    u = nxt[w]
                    emitted_proj.add(u)
                    add_group_front(("kq", u), kq_items(u))
                    add_group(("v", u), v_items(u))
                fill(fills[w])
                if i > 0:
                    wp, pp = pairs[i - 1]
                    # barrier: PV(wp,pp) reads v_aug blocks 2pp,2pp+1
                    vneed = (2 * pp + 1) // 4
                    flush(lambda t: t[0] == "v" and t[1] <= vneed)
                    emit_pv(wp, pp, pts, pouts)
                    if pp == 2 * wp + 1:  # closed window wp
                        box = out_evac(wp, pouts)
                        add_group(("out",), out_items(wp, box))
                    elif (wp, pp) == (3, 6):
                        # cols 0:256 of window 3's out^T are final after
                        # pair 6 (pair 7's diagonal blocks only touch
                        # 256:512): ship them before the last pair so the
                        # end-of-kernel DMA+receipt only covers half
                        oT3 = out_pool.tile([H + 1, W], f32, tag="oT", name="oT3")
                        nc.vector.tensor_copy(oT3[:, 0:256], pouts[3][:, 0:256])
                        nc.sync.dma_start(out=Y[:, 3, 0:256], in_=oT3[:, 0:256])
            wl, pl = pairs[-1]
            flush(lambda t: t[0] == "v" and t[1] <= (2 * pl + 1) // 4)
            emit_pv(wl, pl, pts, pouts)
            flush(lambda t: True)
            nc.vector.tensor_copy(oT3[:, 256:512], pouts.pop(wl)[:, 256:512])
            nc.sync.dma_start(out=Y[:, wl, 256:512], in_=oT3[:, 256:512])

    nc.finalize()
    return nc


def _np_dt(name):
    if name == "bf16":
        import ml_dtypes

        return ml_dtypes.bfloat16
    return np.float32


def _host_inputs(X, Wk, Wq, Wv):
    """Per-core input dicts (host-side sharding + layout prep)."""
    xnp = _np_dt(XDT_NAME)
    scale = 1.0 / np.sqrt(np.float32(D))
    wkq = np.concatenate([Wk, Wq * scale], axis=0).T  # [D, 128]
    wkq = np.ascontiguousarray(
        wkq.reshape(NC_, 128, 128).transpose(1, 0, 2)
    ).astype(xnp)  # [p, c, m]
    wv = np.ascontiguousarray(
        Wv.T.reshape(NC_, 128, H).transpose(1, 0, 2)
    ).astype(xnp)  # [p, c, h]
    pnp = _np_dt(PDT_NAME)
    idt = np.eye(128, dtype=np.float32).astype(pnp)

    in_maps = []
    for b in range(N_CORES):
        xt = np.ascontiguousarray(
            X[b].T.reshape(NC_, 128, NW, W).transpose(1, 2, 0, 3)
        ).astype(xnp)  # [p, w, c, s]
        in_maps.append({"XT": xt, "WKQ": wkq, "WV": wv, "IDT": idt})
    return in_maps


def _unshard(results):
    """Device Y is unnormalized out^T [h(+denom), w, s]; divide + transpose."""
    outs = []
    for i in range(N_CORES):
        y = results[i]["Y"]  # [65, NW, W] f32
        o = y[:H] / y[H : H + 1]  # [64, 4, 512]
        outs.append(np.ascontiguousarray(o.transpose(1, 2, 0)).reshape(S, H))
    return np.stack(outs, axis=0).astype(np.float32)


def kernel(X, Wk, Wq, Wv):
    global _compiled
    from concourse.bass_utils import run_bass_kernel_spmd

    if _compiled is None:
        _compiled = _build()
    in_maps = _host_inputs(
        np.asarray(X, dtype=np.float32),
        np.asarray(Wk, dtype=np.float32),
        np.asarray(Wq, dtype=np.float32),
        np.asarray(Wv, dtype=np.float32),
    )
    res = run_bass_kernel_spmd(_compiled, in_maps, list(range(N_CORES)))
    return _unshard(res.results)
